# revision 27
# baseline (speedup 1.0000x reference)
"""Trainium2 Bass kernel for nn_Decoder_4561255269164 (retrieval_knn).

Math: the reference's top-K(8) KNN collapses to min-reductions:
  - backward: weight w=1/sqrt(d) is nonzero only where d equals the row min
    (over kept candidates), so the scatter-add num/den equals
    E_b^T @ [w*rgb, w] with E_b[i,j] = (d2[i,j] <= rowmin_i*(1+1e-6)).
  - forward: only the column argmin rows of d2 matter; sumf/cntf =
    E_f^T @ [rgb, 1] with E_f[i,j] = (d2[i,j] <= colmin_j*(1+1e-6)).
  - exact-match (d==0) rows use a separate weight column gated by rowmin==0.

Key optimizations vs the first working version:
  - only KEPT candidates (exactly points_num=8192 of 16384 for this input)
    ever reach the loss; the host compacts candidate planes to the kept set,
    halving all device work (pad-tolerant: falls back to 9216 if ever >8192).
  - all distance matmuls run as float32r (1 cycle/row for >=256-wide moving
    dim vs 4 for float32), via AP.bitcast; indicators compare directly
    against PSUM f32 with a 1e-6 relative margin (drops the relu+bf16
    roundtrip of d2 tiles entirely).
  - column-min pass (AC) runs first so the AllReduce(min) overlaps the
    row-min pass (AT).

Sharding: targets (N) split across cores (padded to NT*128 rows each).
Each core computes partial colmins (AllReduce-min), rowmins for its shard,
then indicator matmuls accumulating [12, KL] partials (AllReduce-add), and a
redundant O(KL) finalize. d2 is computed on the fly by K=5 augmented matmuls
(never stored): s[i,j] = -2*t_i.c_j + a2_i + b2_j; pad columns get b2=1e30.
"""

import numpy as np

import concourse.bass as bass
import concourse.bacc as bacc
import concourse.mybir as mybir
import concourse.tile as tile
from concourse import library_config
from concourse.bass_utils import run_bass_kernel_spmd

F32 = mybir.dt.float32
F32R = mybir.dt.float32r
BF16 = mybir.dt.bfloat16
AX = mybir.AxisListType
ALU = mybir.AluOpType
ACTF = mybir.ActivationFunctionType

# geometry
NCORES = 8
L = 16384          # full candidate count (BCE runs over all of L)
N = 10000          # targets
NT = 10            # i-tiles of 128 per core (pad 1250 -> 1280)
NPAD = NT * 128
POINTS_NUM = 8192
BW = 1024          # candidate chunk width for AT/B passes
BIG = np.float32(1e30)
FWD_EPS = 1.0005  # ~half bf16 ulp: threshold rounds up, never down

DEF_PHASES = ("AC", "ARM", "AT", "W", "B", "ARN", "FIN")

# engine-routing knobs (tuned via TimelineSim)
AT_SCE_JC = tuple(range(8))   # AT chunks converted on ScE (rest: DVE f32)
AC_DVE_MOD = 0                # jt%8 < this -> DVE-direct reduce
EF_GP = False                 # ef compare on gpsimd (else DVE)
AC_GP_FOLD = False            # AC ScE-path fold on gpsimd (else DVE)


def _r(ap):
    return ap.bitcast(F32R)


def _build_nc(reps=1, phases=DEF_PHASES, kl=POINTS_NUM):
    assert kl % BW == 0 and kl % 128 == 0
    njt = kl // 128           # 128-wide candidate tiles (AC)
    nch = kl // BW            # 1024-wide candidate chunks (AT/B)
    ngB = BW // 512           # 2
    ngC = (NPAD + 511) // 512  # 3
    ngR = max(ngB, ngC)       # SBUF replicas for tile_position offsets
    nq = BW // 512
    lp = L // 128             # full-L plane width (BCE)
    kp = kl // 128            # kept plane width

    nc = bacc.Bacc("TRN2", target_bir_lowering=False, debug=False,
                   num_devices=NCORES)

    c5d = nc.declare_dram_parameter("c5", [5, kl], F32R, isOutput=False)
    t5d = nc.declare_dram_parameter("t5", [5, NPAD], F32R, isOutput=False)
    trgbd = nc.declare_dram_parameter("trgb", [128, NT * 3], F32,
                                      isOutput=False)
    rgbpd = nc.declare_dram_parameter("rgbp", [3, kl], F32, isOutput=False)
    keepd = nc.declare_dram_parameter("keepf", [1, kl], F32, isOutput=False)
    predd = nc.declare_dram_parameter("predf", [1, L], F32, isOutput=False)
    ktgtd = nc.declare_dram_parameter("ktgt", [1, L], F32, isOutput=False)
    eyed = nc.declare_dram_parameter("eye128", [128, 128], F32,
                                     isOutput=False)
    chaind = nc.declare_dram_parameter("chain", [1, 2], F32, isOutput=False)
    outd = nc.declare_dram_parameter("out", [1, 2], F32, isOutput=True)

    rg = [list(range(NCORES))]

    with tile.TileContext(nc) as tc:
        nc.gpsimd.load_library(library_config.mlp)
        for _rep in range(reps):
            with (
                tc.tile_pool(name="persist", bufs=1) as pp,
                tc.tile_pool(name="dram", bufs=1, space="DRAM") as dp,
            ):
                t5s4 = pp.tile([(ngR - 1) * 32 + 5, NPAD], F32R, tag="t5s4",
                               name="t5s4")
                for g in range(ngR):
                    nc.sync.dma_start(t5s4[32 * g:32 * g + 5, :], t5d[:, :])
                c5s = pp.tile([(ngR - 1) * 32 + 5, kl], F32R, tag="c5s",
                              name="c5s")
                for g in range(ngR):
                    nc.sync.dma_start(c5s[32 * g:32 * g + 5, :], c5d[:, :])
                trgb = eye = None

                if "W" in phases:
                    trgb = pp.tile([128, NT * 3], F32, tag="trgb", name="trgb")
                    nc.sync.dma_start(trgb[:], trgbd[:, :])
                if "AC" in phases:
                    eye = pp.tile([128, 128], F32, tag="eye", name="eye")
                    nc.sync.dma_start(eye[:], eyed[:, :])

                m_all = pp.tile([128, NT], F32, tag="m_all")   # row mins (>=0)
                wb_all = pp.tile([128, NT * 8], BF16, tag="wb_all")
                wf_all = pp.tile([128, NT * 4], BF16, tag="wf_all")
                m2loc = pp.tile([128, njt], F32, tag="m2loc")  # [p, jt]

                m2_in = dp.tile([njt, 128], F32, tag="m2_in")   # j-linear
                m2_out = dp.tile([1, kl], F32, tag="m2_out")
                nd_in = dp.tile([12, kl], F32, tag="nd_in")
                nd_out = dp.tile([12, kl], F32, tag="nd_out")

                if {"AC", "AT"} & set(phases):
                    # ---- interleaved col-min (AC) + row-min (AT) passes ----
                    # Each "unit" is one psum tile; min-extraction is split
                    # between DVE (f32 direct, no conversion) and ScE+DVE
                    # (relu->bf16 then 2x-mode min) to balance both engines.
                    # AC units are front-loaded 2-per-AT-chunk so the
                    # AllReduce(min) fires early and hides under the AT tail.
                    do_ac = "AC" in phases
                    do_at = "AT" in phases
                    with (
                        tc.tile_pool(name="at_ps", bufs=2, space="PSUM") as psp,
                        tc.tile_pool(name="ac_ps", bufs=1, space="PSUM") as psp2,
                        tc.tile_pool(name="tr_ps", bufs=1, space="PSUM") as trp,
                        tc.tile_pool(name="at_db", bufs=2) as atdb,
                        tc.tile_pool(name="ac_db", bufs=2) as acdb,
                        tc.tile_pool(name="at_m", bufs=4) as atm,
                        tc.tile_pool(name="at_f", bufs=2) as atf,
                        tc.tile_pool(name="ac_f", bufs=2) as acf,
                        tc.tile_pool(name="tr_sb", bufs=1) as trs,
                    ):
                        def emit_ac(jt):
                            ps = psp2.tile([128, NPAD], F32, tag="ps2")
                            for g in range(ngC):
                                q0 = g * 512
                                qw = min(512, NPAD - q0)
                                nc.tensor.matmul(
                                    ps[:, q0:q0 + qw],
                                    lhsT=(c5s[32 * g:32 * g + 5,
                                              jt * 128:(jt + 1) * 128]),
                                    rhs=(t5s4[32 * g:32 * g + 5, q0:q0 + qw]),
                                    start=True, stop=True,
                                    tile_position=(32 * g, 0))
                            if jt % 8 < AC_DVE_MOD:
                                # DVE-direct: raw f32 reduce (relu at m2t)
                                nc.vector.tensor_reduce(
                                    m2loc[:, jt:jt + 1], ps[:],
                                    axis=AX.X, op=ALU.min)
                            else:
                                # ScE converts; gpsimd folds + reduces
                                d2c = acdb.tile([128, NPAD], BF16, tag="d2c")
                                nc.scalar.activation(d2c[:], ps[:], ACTF.Relu)
                                half = NPAD // 2
                                mf = acf.tile([128, half], BF16, tag="mf")
                                eng = nc.gpsimd if AC_GP_FOLD else nc.vector
                                eng.tensor_tensor(
                                    mf[:], d2c[:, 0:half], d2c[:, half:NPAD],
                                    op=ALU.min)
                                nc.vector.tensor_reduce(
                                    m2loc[:, jt:jt + 1], mf[:],
                                    axis=AX.X, op=ALU.min)

                        def emit_arm():
                            # transpose m2loc -> j-linear, relu+margin, then
                            # AllReduce(min).
                            pst = trp.tile([128, 128], F32, tag="pst")
                            nc.tensor.transpose(pst[0:njt, :], m2loc[:],
                                                eye[:])
                            m2t = trs.tile([njt, 128], F32, tag="m2t")
                            nc.vector.tensor_scalar(m2t[:], pst[0:njt, :],
                                                    0.0, float(FWD_EPS),
                                                    op0=ALU.max, op1=ALU.mult)
                            nc.sync.dma_start(m2_in[:, :], m2t[:])
                            if "ARM" in phases and NCORES > 1:
                                nc.gpsimd.collective_compute(
                                    "AllReduce", ALU.min, replica_groups=rg,
                                    ins=[m2_in.opt()], outs=[m2_out.opt()])
                            else:
                                nc.sync.dma_start(
                                    m2_out[0, :],
                                    m2_in[:, :].rearrange("a b -> (a b)"))

                        ac_pos = 0
                        arm_done = False

                        def pump_ac(k):
                            nonlocal ac_pos, arm_done
                            for _ in range(k):
                                if do_ac and ac_pos < njt:
                                    emit_ac(ac_pos)
                                    ac_pos += 1
                            if do_ac and ac_pos >= njt and not arm_done:
                                emit_arm()
                                arm_done = True

                        SCE_JC = tuple(AT_SCE_JC)
                        dve_jc = tuple(j for j in range(nch)
                                       if j not in SCE_JC)
                        for t in range(NT if do_at else 0):
                            macc_f = (atm.tile([128, BW], F32, tag="maccf",
                                                name="maccf")
                                      if dve_jc else None)
                            macc_b = (atm.tile([128, BW], BF16, tag="maccb",
                                                name="maccb")
                                      if SCE_JC else None)
                            for jc in range(nch):
                                ps = psp.tile([128, BW], F32, tag="ps")
                                for g in range(ngB):
                                    q0 = g * 512
                                    nc.tensor.matmul(
                                        ps[:, q0:q0 + 512],
                                        lhsT=(t5s4[32 * g:32 * g + 5,
                                                   t * 128:(t + 1) * 128]),
                                        rhs=(c5s[32 * g:32 * g + 5,
                                                 jc * BW + q0:
                                                 jc * BW + q0 + 512]),
                                        start=True, stop=True,
                                        tile_position=(32 * g, 0))
                                if jc in SCE_JC:
                                    d2b = atdb.tile([128, BW], BF16,
                                                    tag="d2b")
                                    nc.scalar.activation(d2b[:], ps[:],
                                                         ACTF.Relu)
                                    if jc == SCE_JC[0]:
                                        nc.vector.tensor_copy(macc_b[:],
                                                              d2b[:])
                                    else:
                                        nc.vector.tensor_tensor(
                                            macc_b[:], macc_b[:], d2b[:],
                                            op=ALU.min)
                                else:
                                    if jc == dve_jc[0]:
                                        nc.vector.tensor_copy(macc_f[:],
                                                              ps[:])
                                    else:
                                        nc.vector.tensor_tensor(
                                            macc_f[:], macc_f[:], ps[:],
                                            op=ALU.min)
                                # (f32 path: psum touch and min in one op)
                                pump_ac(2)
                            # combine: bf16(relu(min f32 chunks)) min bf16 acc
                            if dve_jc:
                                mcb = atdb.tile([128, BW], BF16, tag="mcb")
                                nc.scalar.activation(mcb[:], macc_f[:],
                                                     ACTF.Relu)
                                if SCE_JC:
                                    nc.vector.tensor_tensor(
                                        mcb[:], mcb[:], macc_b[:], op=ALU.min)
                            else:
                                mcb = macc_b
                            half = BW // 2
                            mfat = atf.tile([128, half], BF16, tag="mfat")
                            nc.vector.tensor_tensor(mfat[:], mcb[:, 0:half],
                                                    mcb[:, half:BW],
                                                    op=ALU.min)
                            nc.vector.tensor_reduce(
                                m_all[:, t:t + 1], mfat[:], axis=AX.X,
                                op=ALU.min)
                        pump_ac(njt)  # drain if AT disabled


                if "W" in phases:
                    # ---------------- weight tiles ----------------------------
                    with tc.tile_pool(name="wsmall", bufs=1) as ws:
                        m_relu = m_all  # d2b tiles are relu'd already
                        msafe = ws.tile([128, NT], F32, tag="msafe")
                        nc.vector.tensor_scalar(msafe[:], m_relu[:], 1e-30,
                                                None, op0=ALU.max)
                        sqm = ws.tile([128, NT], F32, tag="sqm")
                        nc.scalar.activation(sqm[:], msafe[:], ACTF.Sqrt)
                        w0 = ws.tile([128, NT], F32, tag="w0")
                        nc.vector.reciprocal(w0[:], sqm[:])
                        vv = ws.tile([128, NT], F32, tag="vv")
                        nc.vector.tensor_scalar(vv[:], m_relu[:], 0.0, None,
                                                op0=ALU.is_gt)
                        v2 = ws.tile([128, NT], F32, tag="v2")
                        nc.vector.tensor_scalar(v2[:], m_relu[:], 1e29, None,
                                                op0=ALU.is_lt)
                        nc.vector.tensor_tensor(vv[:], vv[:], v2[:],
                                                op=ALU.mult)
                        wgt = ws.tile([128, NT], F32, tag="wgt")
                        nc.vector.tensor_tensor(wgt[:], w0[:], vv[:],
                                                op=ALU.mult)
                        zz = ws.tile([128, NT], F32, tag="zz")
                        nc.vector.tensor_scalar(zz[:], m_relu[:], 0.0, None,
                                                op0=ALU.is_equal)

                        wbv = wb_all[:].rearrange("p (t k) -> p t k", k=8)
                        wfv = wf_all[:].rearrange("p (t k) -> p t k", k=4)
                        tv = trgb[:].rearrange("p (t k) -> p t k", k=3)
                        wgv = wgt[:].rearrange("p (t o) -> p t o", o=1)
                        zzv = zz[:].rearrange("p (t o) -> p t o", o=1)
                        for c in range(3):
                            nc.vector.tensor_tensor(
                                wbv[:, :, c:c + 1], wgv, tv[:, :, c:c + 1],
                                op=ALU.mult)
                            nc.vector.tensor_tensor(
                                wbv[:, :, 4 + c:5 + c], zzv, tv[:, :, c:c + 1],
                                op=ALU.mult)
                            nc.vector.tensor_copy(wfv[:, :, c:c + 1],
                                                  tv[:, :, c:c + 1])
                        nc.vector.tensor_copy(wbv[:, :, 3:4], wgv)
                        nc.vector.tensor_copy(wbv[:, :, 7:8], zzv)
                        nc.vector.memset(wfv[:, :, 3:4], 1.0)

                # prefetch FIN planes that do not depend on the collectives
                fin_pre = {}
                if "FIN" in phases:
                    fpp = pp
                    def plane_pre(dram_row, tg, width):
                        tl = fpp.tile([128, width], F32, tag=tg, name=tg)
                        nc.sync.dma_start(
                            tl[:], dram_row.rearrange("(p q) -> p q", p=128))
                        return tl
                    for k in range(3):
                        fin_pre[f"rgb{k}"] = plane_pre(rgbpd[k, :], f"rgb{k}",
                                                       kp)
                    fin_pre["keepf"] = plane_pre(keepd[0, :], "keepf", kp)
                    fin_pre["predf"] = plane_pre(predd[0, :], "predf", lp)
                    fin_pre["ktgt"] = plane_pre(ktgtd[0, :], "ktgt", lp)

                    # BCE: relu(p) - p*t + softplus(-|p|) over full L.
                    # Emitted before B so it drains on otherwise-idle
                    # scalar/vector cycles; only bce_rows survives to FIN.
                    bce_rows = pp.tile([128, 1], F32, tag="bce_rows")
                    with tc.tile_pool(name="bcep", bufs=1) as bp_:
                        predf = fin_pre["predf"]
                        ktgt = fin_pre["ktgt"]
                        bce = bp_.tile([128, lp], F32, tag="bce")
                        nc.scalar.activation(bce[:], predf[:], ACTF.Relu)
                        pt = bp_.tile([128, lp], F32, tag="pt")
                        nc.vector.tensor_tensor(pt[:], predf[:], ktgt[:],
                                                op=ALU.mult)
                        nc.vector.tensor_tensor(bce[:], bce[:], pt[:],
                                                op=ALU.subtract)
                        ap_ = bp_.tile([128, lp], F32, tag="ap_")
                        nc.scalar.activation(ap_[:], predf[:], ACTF.Abs)
                        en = bp_.tile([128, lp], F32, tag="en")
                        nc.scalar.activation(en[:], ap_[:], ACTF.Exp,
                                             scale=-1.0)
                        sp = bp_.tile([128, lp], F32, tag="sp")
                        nc.scalar.activation(sp[:], en[:], ACTF.Ln, bias=1.0)
                        nc.vector.tensor_tensor(bce[:], bce[:], sp[:],
                                                op=ALU.add)
                        nc.vector.tensor_reduce(bce_rows[:], bce[:],
                                                axis=AX.X, op=ALU.add)

                if "B" in phases:
                    # ---------------- Pass B: indicators + scatter matmuls ----
                    with (
                        tc.tile_pool(name="b_m2r", bufs=2) as bm2,
                        tc.tile_pool(name="b_m2b", bufs=2) as bm2b,
                        tc.tile_pool(name="b_e", bufs=6) as bep,
                        tc.tile_pool(name="b_psd", bufs=2, space="PSUM") as bpsd,
                        tc.tile_pool(name="b_acc", bufs=1, space="PSUM") as baccp,
                    ):
                        for jc in range(nch):
                            m2b = bm2b.tile([128, BW], BF16, tag="m2b")
                            m2rw = bm2.tile([1, BW], F32, tag="m2rw")
                            nc.sync.dma_start(
                                m2rw[:],
                                m2_out[:, jc * BW:(jc + 1) * BW])
                            m2rwb = bm2.tile([1, BW], BF16, tag="m2rwb")
                            nc.vector.tensor_copy(m2rwb[:], m2rw[:])
                            nc.gpsimd.partition_broadcast(m2b[:], m2rwb[:])

                            accb = [baccp.tile([8, 512], F32, tag=f"accb{q}",
                                               name=f"accb{q}")
                                    for q in range(nq)]
                            accf = [baccp.tile([4, 512], F32, tag=f"accf{q}",
                                               name=f"accf{q}")
                                    for q in range(nq)]

                            # software-pipelined: emit d2(t+1) before acc(t)
                            # so the tensor queue never stalls on the DVE
                            # indicator ops.
                            def emit_d2(t):
                                psd = bpsd.tile([128, BW], F32, tag="psd")
                                for g in range(ngB):
                                    q0 = g * 512
                                    nc.tensor.matmul(
                                        psd[:, q0:q0 + 512],
                                        lhsT=(t5s4[32 * g:32 * g + 5,
                                                   t * 128:(t + 1) * 128]),
                                        rhs=(c5s[32 * g:32 * g + 5,
                                                 jc * BW + q0:
                                                 jc * BW + q0 + 512]),
                                        start=True, stop=True,
                                        tile_position=(32 * g, 0))
                                return psd

                            psd_next = emit_d2(0)
                            for t in range(NT):
                                psd = psd_next
                                d2b = bep.tile([128, BW], BF16, tag="d2b")
                                nc.scalar.activation(d2b[:], psd[:], ACTF.Relu)
                                eb = bep.tile([128, BW], BF16, tag="eb")
                                nc.vector.tensor_scalar(
                                    eb[:], d2b[:], m_all[:, t:t + 1],
                                    None, op0=ALU.is_equal)
                                ef = bep.tile([128, BW], BF16, tag="ef")
                                efe = nc.gpsimd if EF_GP else nc.vector
                                efe.tensor_tensor(ef[:], d2b[:],
                                                  m2b[:], op=ALU.is_le)
                                if t + 1 < NT:
                                    psd_next = emit_d2(t + 1)
                                for q in range(nq):
                                    nc.tensor.matmul(
                                        accb[q][:, :],
                                        lhsT=wb_all[:, t * 8:(t + 1) * 8],
                                        rhs=eb[:, q * 512:(q + 1) * 512],
                                        start=(t == 0), stop=(t == NT - 1))
                                for q in range(nq):
                                    nc.tensor.matmul(
                                        accf[q][:, :],
                                        lhsT=wf_all[:, t * 4:(t + 1) * 4],
                                        rhs=ef[:, q * 512:(q + 1) * 512],
                                        start=(t == 0), stop=(t == NT - 1))
                            for q in range(nq):
                                j0 = jc * BW + q * 512
                                ndsb = bep.tile([36, 512], F32, tag="ndsb",
                                                name="ndsb")
                                nc.scalar.copy(ndsb[0:8, :], accb[q][:, :])
                                nc.scalar.copy(ndsb[32:36, :], accf[q][:, :])
                                nc.sync.dma_start(nd_in[0:8, j0:j0 + 512],
                                                  ndsb[0:8, :])
                                nc.sync.dma_start(nd_in[8:12, j0:j0 + 512],
                                                  ndsb[32:36, :])
                    if "ARN" in phases and NCORES > 1:
                        nc.gpsimd.collective_compute(
                            "AllReduce", ALU.add, replica_groups=rg,
                            ins=[nd_in.opt()], outs=[nd_out.opt()])
                    else:
                        nc.sync.dma_start(nd_out[:, :], nd_in[:, :])

                if "FIN" in phases:
                    # ---------------- finalize (redundant on every core) ------
                    with (
                        tc.tile_pool(name="fin", bufs=1) as fp,
                        tc.tile_pool(name="fin_ps", bufs=1, space="PSUM") as fps,
                    ):
                        def plane_from(dram_row, tg):
                            tl = fp.tile([128, kp], F32, tag=tg, name=tg)
                            nc.sync.dma_start(
                                tl[:], dram_row.rearrange("(p q) -> p q",
                                                          p=128))
                            return tl

                        nd = [plane_from(nd_out[k, :], f"nd{k}")
                              for k in range(12)]
                        rgbp = [fin_pre[f"rgb{k}"] for k in range(3)]
                        keepf = fin_pre["keepf"]

                        num, den = nd[0:3], nd[3]
                        s0, cnt0 = nd[4:7], nd[7]
                        sf, cntf = nd[8:11], nd[11]

                        _cnt = [0]

                        def newt(width=kp):
                            _cnt[0] += 1
                            return fp.tile([128, width], F32,
                                           tag=f"fin{_cnt[0]}",
                                           name=f"fin{_cnt[0]}")

                        dsafe = newt()
                        nc.vector.tensor_scalar(dsafe[:], den[:], 0.0, None,
                                                op0=ALU.is_equal)
                        nc.vector.tensor_tensor(dsafe[:], dsafe[:], den[:],
                                                op=ALU.add)
                        rden = newt()
                        nc.vector.reciprocal(rden[:], dsafe[:])
                        c0safe = newt()
                        nc.vector.tensor_scalar(c0safe[:], cnt0[:], 0.0, None,
                                                op0=ALU.is_equal)
                        nc.vector.tensor_tensor(c0safe[:], c0safe[:], cnt0[:],
                                                op=ALU.add)
                        rcnt0 = newt()
                        nc.vector.reciprocal(rcnt0[:], c0safe[:])
                        cfsafe = newt()
                        nc.vector.tensor_scalar(cfsafe[:], cntf[:], 0.0, None,
                                                op0=ALU.is_equal)
                        nc.vector.tensor_tensor(cfsafe[:], cfsafe[:], cntf[:],
                                                op=ALU.add)
                        rcntf = newt()
                        nc.vector.reciprocal(rcntf[:], cfsafe[:])

                        mden = fp.tile([128, kp], mybir.dt.int32, tag="mden",
                                       name="mden")
                        nc.vector.tensor_scalar(mden[:], den[:], 0.0, None,
                                                op0=ALU.not_equal)
                        mz = fp.tile([128, kp], mybir.dt.int32, tag="mz",
                                     name="mz")
                        nc.vector.tensor_scalar(mz[:], cnt0[:], 0.0, None,
                                                op0=ALU.is_gt)

                        acc = newt()
                        nc.vector.memset(acc[:], 0.0)
                        for c in range(3):
                            rec = newt()
                            nc.vector.tensor_tensor(rec[:], sf[c][:], rcntf[:],
                                                    op=ALU.mult)
                            tmp = newt()
                            nc.vector.tensor_tensor(tmp[:], num[c][:], rden[:],
                                                    op=ALU.mult)
                            nc.vector.copy_predicated(rec[:], mden[:], tmp[:])
                            nc.vector.tensor_tensor(tmp[:], s0[c][:], rcnt0[:],
                                                    op=ALU.mult)
                            nc.vector.copy_predicated(rec[:], mz[:], tmp[:])
                            diff = newt()
                            nc.vector.tensor_tensor(diff[:], rgbp[c][:],
                                                    rec[:], op=ALU.subtract)
                            ad = newt()
                            nc.scalar.activation(ad[:], diff[:], ACTF.Abs)
                            nc.vector.tensor_tensor(acc[:], acc[:], ad[:],
                                                    op=ALU.add)
                        nc.vector.tensor_tensor(acc[:], acc[:], keepf[:],
                                                op=ALU.mult)

                        rows2 = fp.tile([128, 2], F32, tag="rows2")
                        nc.vector.tensor_copy(rows2[:, 0:1], bce_rows[:])
                        nc.vector.tensor_reduce(rows2[:, 1:2], acc[:],
                                                axis=AX.X, op=ALU.add)
                        onescol = fp.tile([128, 1], F32, tag="onescol")
                        nc.vector.memset(onescol[:], 1.0)
                        pstot = fps.tile([1, 2], F32, tag="pstot")
                        nc.tensor.matmul(pstot[:], lhsT=onescol[:],
                                         rhs=rows2[:], start=True, stop=True)
                        chsb = fp.tile([1, 2], F32, tag="chsb")
                        nc.sync.dma_start(chsb[:], chaind[:, :])
                        nc.vector.tensor_scalar(chsb[:], chsb[:], 0.0, None,
                                                op0=ALU.mult)
                        outsb = fp.tile([1, 2], F32, tag="outsb")
                        nc.scalar.copy(outsb[:], pstot[:])
                        nc.vector.tensor_tensor(outsb[:], outsb[:], chsb[:],
                                                op=ALU.add)
                        nc.sync.dma_start(outd[:, :], outsb[:])

    nc.compile()
    return nc


def _host_prep(pred_F, cand_xyz, cand_rgb, tgt_xyz, tgt_rgb, keep_target,
               points_num):
    nsh = N // NCORES
    pred = np.ascontiguousarray(np.asarray(pred_F, np.float32))
    cxyz = np.ascontiguousarray(np.asarray(cand_xyz, np.float32))
    crgb = np.ascontiguousarray(np.asarray(cand_rgb, np.float32))
    txyz = np.ascontiguousarray(np.asarray(tgt_xyz, np.float32))
    trgb_np = np.ascontiguousarray(np.asarray(tgt_rgb, np.float32))
    ktgt = np.asarray(keep_target).astype(np.float32)

    # keep mask (exact reference semantics, f32)
    p8 = pred.reshape(-1, 8)
    rows = np.arange(p8.shape[0])
    ilm = np.zeros(p8.shape, dtype=bool)
    ilm[rows, np.argmax(p8, axis=1)] = True
    ilm = ilm.reshape(-1)
    k = L - int(points_num)
    vals = np.where(ilm, np.inf, pred)
    thr = np.sort(vals)[k - 1]
    keep = (pred > thr) | ilm

    idx = np.nonzero(keep)[0]
    nk = len(idx)
    kl = POINTS_NUM if nk <= POINTS_NUM else POINTS_NUM + 1024
    assert nk <= kl

    cK = np.zeros((kl, 3), np.float32)
    cK[:nk] = cxyz[idx]
    b2 = np.sum(cK * cK, axis=1, dtype=np.float32).astype(np.float32)
    b2[nk:] = BIG
    ones = np.ones(kl, np.float32)
    c5 = np.ascontiguousarray(
        np.stack([cK[:, 0], cK[:, 1], cK[:, 2], ones, b2]))

    a2 = np.sum(txyz * txyz, axis=1, dtype=np.float32).astype(np.float32)

    t5_cores, trgb_cores = [], []
    for c in range(NCORES):
        sl = slice(c * nsh, (c + 1) * nsh)
        t5 = np.zeros((5, NPAD), np.float32)
        t5[3, :] = BIG     # pad rows: s = 1e30 everywhere
        t5[4, :] = 1.0
        t5[0, :nsh] = -2.0 * txyz[sl, 0]
        t5[1, :nsh] = -2.0 * txyz[sl, 1]
        t5[2, :nsh] = -2.0 * txyz[sl, 2]
        t5[3, :nsh] = a2[sl]
        tr = np.zeros((NPAD, 3), np.float32)
        tr[:nsh] = trgb_np[sl]
        # [p, t*3+c] layout: target i_local = t*128 + p
        trc = tr.reshape(NT, 128, 3).transpose(1, 0, 2).reshape(128, NT * 3)
        t5_cores.append(np.ascontiguousarray(t5))
        trgb_cores.append(np.ascontiguousarray(trc))

    rgbp = np.zeros((3, kl), np.float32)
    rgbp[:, :nk] = (crgb[idx] * np.float32(255.0)).T
    keepf = np.zeros((1, kl), np.float32)
    keepf[0, :nk] = 1.0
    eye = np.eye(128, dtype=np.float32)

    common = dict(c5=c5, rgbp=np.ascontiguousarray(rgbp),
                  keepf=keepf, predf=pred.reshape(1, L),
                  ktgt=ktgt.reshape(1, L), eye128=eye,
                  chain=np.zeros((1, 2), np.float32))
    in_maps = [dict(common, t5=t5_cores[c], trgb=trgb_cores[c])
               for c in range(NCORES)]
    return in_maps, kl


_CACHE = {}


def kernel(pred_F, cand_xyz, cand_rgb, tgt_xyz, tgt_rgb, keep_target,
           points_num=8192, **_ignored):
    in_maps, kl = _host_prep(pred_F, cand_xyz, cand_rgb, tgt_xyz, tgt_rgb,
                             keep_target, points_num)
    if kl not in _CACHE:
        _CACHE[kl] = _build_nc(kl=kl)
    res = run_bass_kernel_spmd(_CACHE[kl], in_maps,
                               core_ids=list(range(NCORES)))
    return np.asarray(res.results[0]["out"], np.float32).reshape(2)


if __name__ == "__main__":
    import reference as R
    inputs = R.setup_inputs()
    inputs = {kk: np.asarray(vv) if not np.isscalar(vv) else vv
              for kk, vv in inputs.items()}
    out = kernel(**inputs)
    print("kernel out:", out)
